# revision 12
# baseline (speedup 1.0000x reference)
"""Trainium2 Bass kernel for nn_Attention_49907519979595 (Bahdanau-style attention).

v5 design (v2 measured 165-172us, PE-bound: PE issue-time 155us =
transposes 55 + wsum 40 + r 31 + lgT 18 + misc; ACT 135 = tanh 71 +
PSUM copies 51 + exp/z 19).

Key change: ref streams from HBM as FP8 (SWDGE cast DMA), not bf16.
  - PE transposes operate on fp8 PAIRS viewed as uint16: 4 transposes per
    512-s tile instead of 8 (PE -27us), and the PSUM->SBUF copy volume
    halves to u16 (DVE-only, ~28-48us total, ACT keeps only tanh/exp).
  - The r-matmul moving operand is the packed refT: fp8 view of the u16
    transpose output with AP [Ki=128, l(stride 1), s(stride 2)], paired
    with a HOST-packed stationary WrT_pk[p, l, m] = Wr[m, 2p+l] so the
    DoubleRow contraction hi = 2*ki + l is consistent.
  - wsum runs DoubleRow too: stationary = e-column PAIRS (exp now emits
    fp8 e_col), moving = nat8[:, j:j+2, :] - 2 matmuls per tile (PE -13us).
  - All weight-only algebra is HOST-side numpy, passed as inputs:
    bias_sb = (Wq q^T + bq + br) chunks, MT = (WoB Wr)^T, and
    res0 = out_prev WoA^T + br WoB^T + bo.  Device epilogue is just
    uT = (wsum/Z)^T via tiny PE transposes + 2 matmuls + res0 add.

Engine budget targets: PE ~85 (r 29 + transposes 21 + lgT 18 + wsum 15),
ACT ~90 (tanh 71 + exp 19), DVE ~45-65, DMA-HBM ~94-116.
Precision: refT/Wr fp8 (logits-only, v2-proven), e_col fp8 (new, ~3%
weight noise -> ~1e-3 on expected; gate is 2e-2).
"""

import os
import sys

import numpy as np

sys.path.insert(0, "/opt/trn_rl_repo")

H = 256
B = 64
S = 4096
N_CORES = 8
B_CORE = B // N_CORES  # 8
S_TILE = 512
N_STILES = S // S_TILE  # 8 tiles per batch
NT = B_CORE * N_STILES  # 64 tiles
NJB = S // 128  # 32 s-chunks of 128 per batch

_nc_cache = {}


def build_nc():
    import concourse.bacc as bacc
    import concourse.tile as tile
    from concourse import masks, mybir

    f32 = mybir.dt.float32
    u16 = mybir.dt.uint16
    bf16 = mybir.dt.bfloat16
    f8 = mybir.dt.float8e4
    AF = mybir.ActivationFunctionType
    PM = mybir.MatmulPerfMode

    from concourse import bass_isa

    nc = bacc.Bacc("TRN2", debug=False)
    ref = nc.dram_tensor("ref", [B_CORE, S, H], f32, kind="ExternalInput").ap()
    # host-precomputed (see kernel()):
    wrt_pk = nc.dram_tensor("wrt_pk", [128, 2, H], f32, kind="ExternalInput").ap()
    bias_pk = nc.dram_tensor("bias_pk", [128, 2, B_CORE], f32, kind="ExternalInput").ap()
    mt_pk = nc.dram_tensor("mt_pk", [128, 2, H], f32, kind="ExternalInput").ap()
    res0 = nc.dram_tensor("res0", [B_CORE, H], f32, kind="ExternalInput").ap()
    V = nc.dram_tensor("V", [H], f32, kind="ExternalInput").ap()
    result = nc.dram_tensor("result", [B_CORE, H], f32, kind="ExternalOutput").ap()

    with tile.TileContext(nc) as tc:
        with (
            tc.tile_pool(name="const", bufs=1) as const,
            tc.tile_pool(name="natp", bufs=7) as natp,
            tc.tile_pool(name="reftp", bufs=3) as reftp,
            tc.tile_pool(name="tanhp", bufs=3) as tanhp,
            tc.tile_pool(name="small", bufs=6) as small,
        ):
            # Kick the first slice of batch 0 before anything else.
            nat0 = natp.tile([128, NJB, H], f8, name="nat", tag="nat")
            ref0 = ref[0].rearrange("(j p) h -> p j h", p=128)
            with nc.allow_low_precision(reason="fp8 ref stream"):
                nc.gpsimd.dma_start(nat0[:, 0:4, :], ref0[:, 0:4, :])

            # ---------------- prologue ----------------
            psum_pro_cm = tc.tile_pool(name="psum_pro", bufs=2, space="PSUM")
            psum_s = psum_pro_cm.__enter__()
            ident = const.tile([128, 128], f32, name="ident")
            masks.make_identity(nc, ident[:])
            ident_bf = const.tile([128, 128], bf16, name="ident_bf")
            nc.vector.tensor_copy(ident_bf[:], ident[:])

            # early HAM warm-up while weight DMAs land
            warm0_ps = psum_s.tile([128, 128], bf16, name="warm0_ps", tag="warm0")
            for _ in range(40):
                nc.tensor.transpose(warm0_ps[:], ident_bf[:], ident_bf[:])

            def load(shape, src_ap, name, eng):
                t = const.tile(shape, f32, name=name)
                eng.dma_start(t[:], src_ap)
                return t

            wrtpk_sb = load([128, 2, H], wrt_pk, "wrtpk_sb", nc.sync)
            wrtpk_f8 = const.tile([128, 2, H], f8, name="wrtpk_f8")
            with nc.allow_low_precision(reason="fp8 r-matmul; logits-only"):
                nc.vector.tensor_copy(wrtpk_f8[:], wrtpk_sb[:])
            bias_sb = load([128, 2, B_CORE], bias_pk, "bias_sb", nc.scalar)
            MT = load([128, 2, H], mt_pk, "MT", nc.sync)
            res0_sb = load([B_CORE, H], res0, "res0_sb", nc.scalar)
            V_f32 = load([128, 2], V.rearrange("(c p) -> p c", p=128), "V_f32", nc.scalar)
            V_col = const.tile([128, 2], f8, name="V_col")
            with nc.allow_low_precision(reason="fp8 logits; softmax-only"):
                nc.vector.tensor_copy(V_col[:], V_f32[:])

            # per-batch streaming outputs
            acc_bh = const.tile([B_CORE, H], f32, name="acc_bh")
            zcol_all = const.tile([128, B_CORE], f32, name="zcol_all")
            z_sb = const.tile([1, B_CORE], f32, name="z_sb")

            psum_pro_cm.__exit__(None, None, None)
            # main-loop PSUM: r 2x2 + refT_pk 2 + lgT 1 + wsum 1 = 8 banks
            psum_r_cm = tc.tile_pool(name="psum_r", bufs=2, space="PSUM")
            psum_r = psum_r_cm.__enter__()
            psum_t_cm = tc.tile_pool(name="psum_t", bufs=2, space="PSUM")
            psum_t = psum_t_cm.__enter__()
            psum_lg_cm = tc.tile_pool(name="psum_lg", bufs=1, space="PSUM")
            psum_lg = psum_lg_cm.__enter__()
            psum_ws_cm = tc.tile_pool(name="psum_ws", bufs=1, space="PSUM")
            psum_ws = psum_ws_cm.__enter__()
            wsum_acc = psum_ws.tile([1, 256], f32, name="wsum_acc", tag="wsacc")

            # ---------------- main loop (software-pipelined emission) ----------------
            st = {}

            def emit_load(b, slices=(NJB,)):
                nat = natp.tile([128, NJB, H], f8, name="nat", tag="nat")
                src = ref[b].rearrange("(j p) h -> p j h", p=128)
                j0 = 0
                with nc.allow_low_precision(reason="fp8 ref stream"):
                    for js in slices:
                        nc.gpsimd.dma_start(
                            nat[:, j0 : j0 + js, :], src[:, j0 : j0 + js, :]
                        )
                        j0 += js
                st[("nat", b)] = nat

            for a, bnd in ((4, 8), (8, 16), (16, 32)):
                with nc.allow_low_precision(reason="fp8 ref stream"):
                    nc.gpsimd.dma_start(nat0[:, a:bnd, :], ref0[:, a:bnd, :])
            st[("nat", 0)] = nat0
            emit_load(1, slices=(16, 16))

            def stage_load(v):
                b, t = divmod(v, N_STILES)
                if t == 0 and b + 2 < B_CORE:
                    emit_load(b + 2)

            # packed transposes: nat8 pair-chunks as u16 [128, 128] each
            def stage_transpose(v):
                b, t = divmod(v, N_STILES)
                nat = st[("nat", b)]
                rt_ps = psum_t.tile([128, 4, 128], bf16, name="rt_ps", tag="rtps")
                for j in range(4):
                    nc.tensor.transpose(
                        rt_ps[:, j, :],
                        nat[:, t * 4 + j, :].bitcast(bf16),
                        ident_bf[:],
                    )
                st[("rtps", v)] = rt_ps

            def stage_copy(v):
                rt_ps = st.pop(("rtps", v))
                refT_pk = reftp.tile([128, 4, 128], bf16, name="refT_pk", tag="refT")
                nc.vector.tensor_copy(refT_pk[:], rt_ps[:])
                st[("refT", v)] = refT_pk

            def stage_r(v):
                # fires on odd v; emits BOTH tiles of the pair hh-outer so
                # one DoubleRow LDWEIGHTS per hh serves two matmuls, but the
                # two tiles get SEPARATE double-buffered PSUM tiles so the
                # next pair's r doesn't WAR-wait on this pair's full tanh.
                if v % 2 == 0:
                    return
                r0 = psum_r.tile([128, 2, S_TILE], f32, name="r_ps", tag="rps")
                r1 = psum_r.tile([128, 2, S_TILE], f32, name="r_ps", tag="rps")
                rps = [r0, r1]
                refTs = [st.pop(("refT", v - 1)), st.pop(("refT", v))]
                movs = [
                    t[:].bitcast(f8).rearrange("p j (s l) -> p l j s", l=2)
                    for t in refTs
                ]
                for hh in range(2):
                    for ti in range(2):
                        nc.tensor.matmul(
                            rps[ti][:, hh, :],
                            wrtpk_f8[:, :, hh * 128 : (hh + 1) * 128],
                            movs[ti][:],
                            start=True,
                            stop=True,
                            perf_mode=PM.DoubleRow,
                        )
                st[("rps", v - 1)] = r0
                st[("rps", v)] = r1

            def stage_tanh(v):
                b, t = divmod(v, N_STILES)
                r_ps = st.pop(("rps", v))
                tanh_sb = tanhp.tile(
                    [128, 2, S_TILE], f8, name="tanh_sb", tag="tanh",
                )
                with nc.allow_low_precision(reason="fp8 tanh; softmax-only"):
                    for hh in range(2):
                        nc.scalar.activation(
                            tanh_sb[:, hh, :],
                            r_ps[:, hh, :],
                            AF.Tanh,
                            bias=bias_sb[:, hh, b : b + 1],
                        )
                st[("tanh", v)] = tanh_sb

            def stage_lg(v):
                tanh_sb = st.pop(("tanh", v))
                ti = v % 2
                if ti == 0:
                    st[("lgT", v // 2)] = psum_lg.tile(
                        [128, 2 * 4], f32, name="lgT_ps", tag="lgT"
                    )
                lgT_ps = st[("lgT", v // 2)]
                for c in range(4):
                    col = ti * 4 + c
                    for hh in range(2):
                        nc.tensor.matmul(
                            lgT_ps[:, col : col + 1],
                            tanh_sb[:, hh, c * 128 : (c + 1) * 128],
                            V_col[:, hh : hh + 1],
                            start=(hh == 0),
                            stop=(hh == 1),
                        )

            def stage_exp(v):
                if v % 2 == 0:
                    return
                b, t = divmod(v, N_STILES)
                lgT_ps = st.pop(("lgT", v // 2))
                e_col = small.tile([128, 2 * 4], f8, name="e_col", tag="e_col", bufs=3)
                zt = small.tile([128, 1], f32, name="zt", tag="zt", bufs=4)
                with nc.allow_low_precision(reason="fp8 softmax weights"):
                    nc.scalar.activation(e_col[:], lgT_ps[:], AF.Exp, accum_out=zt[:])
                if t == 1:
                    nc.vector.tensor_copy(zcol_all[:, b : b + 1], zt[:])
                else:
                    nc.vector.tensor_add(
                        zcol_all[:, b : b + 1], zcol_all[:, b : b + 1], zt[:]
                    )
                st[("e_col", v - 1)] = e_col
                st[("e_col", v)] = e_col

            def stage_wsum(v):
                b, t = divmod(v, N_STILES)
                e_col = st.pop(("e_col", v))
                nat = st[("nat", b)]
                slot = wsum_acc[:]
                for c in range(4):
                    j = t * 4 + c
                    nc.tensor.matmul(
                        slot,
                        e_col[:, (v % 2) * 4 + c : (v % 2) * 4 + c + 1],
                        nat[:, j, :],
                        start=(j == 0),
                        stop=(j == NJB - 1),
                        skip_group_check=True,
                    )
                if t == N_STILES - 1:
                    ws_sb = small.tile([1, H], f32, name="ws_sb", tag="ws_sb", bufs=2)
                    nc.vector.tensor_copy(ws_sb[:], slot)
                    nc.sync.dma_start(acc_bh[b : b + 1, :], ws_sb[:])
                    zred = small.tile([128, 1], f32, name="zred", tag="zred", bufs=2)
                    nc.gpsimd.partition_all_reduce(
                        zred[:], zcol_all[:, b : b + 1], channels=128,
                        reduce_op=bass_isa.ReduceOp.add,
                    )
                    nc.vector.tensor_copy(z_sb[0:1, b : b + 1], zred[0:1, :])
                    st.pop(("nat", b))

            # exp emitted before tanh AND before lg: exp(pair p) must enter
            # the ACT queue ahead of newer tanhs (latency), and before
            # lg(pair p+1) allocates the bufs=1 lgT slot (WAR tracking).
            # wsum trails exp by 3 steps so the ~5us r->tanh->lg->exp chain
            # latency is absorbed by pipelining instead of stalling PE.
            STAGES = [
                (stage_load, 0),
                (stage_copy, 1),
                (stage_transpose, 0),
                (stage_r, 2),
                (stage_exp, 5),
                (stage_tanh, 3),
                (stage_lg, 4),
                (stage_wsum, 8),
            ]
            LOOKAHEAD = 9
            for step in range(NT + LOOKAHEAD):
                for fn, off in STAGES:
                    w = step - off
                    if 0 <= w < NT:
                        fn(w)

            psum_ws_cm.__exit__(None, None, None)
            psum_lg_cm.__exit__(None, None, None)
            psum_t_cm.__exit__(None, None, None)
            psum_r_cm.__exit__(None, None, None)

            # ---------------- epilogue ----------------
            psum_epi_cm = tc.tile_pool(name="psum_epi", bufs=2, space="PSUM")
            psum_s = psum_epi_cm.__enter__()

            # Z: z_sb [1, 8] -> zrow [8, 1] via one PE transpose
            zt_ps = psum_s.tile([B_CORE, 1], f32, name="zt_ps", tag="ps")
            nc.tensor.transpose(zt_ps[:], z_sb[:], ident[0:1, 0:1])
            zrow = small.tile([B_CORE, 1], f32, name="zrow")
            nc.vector.tensor_copy(zrow[:], zt_ps[:])
            rz = small.tile([B_CORE, 1], f32, name="rz")
            nc.vector.reciprocal(rz[:], zrow[:])

            # u = acc / Z, then transpose to [128, 2, B]
            u_bh = small.tile([B_CORE, H], f32, name="u_bh")
            nc.vector.tensor_scalar_mul(u_bh[:], acc_bh[:], rz[:])
            uT = small.tile([128, 2, B_CORE], f32, name="uT")
            for c in range(2):
                ut_ps = psum_s.tile([128, B_CORE], f32, name="ut_ps", tag="ps")
                nc.tensor.transpose(
                    ut_ps[:], u_bh[:, c * 128 : (c + 1) * 128], ident[:B_CORE, :B_CORE]
                )
                nc.vector.tensor_copy(uT[:, c, :], ut_ps[:])

            # res = res0 + uT.T @ MT
            res_ps = psum_s.tile([B_CORE, H], f32, name="res_ps", tag="ps")
            for ck in range(2):
                nc.tensor.matmul(
                    res_ps[:], uT[:, ck, :], MT[:, ck, :],
                    start=(ck == 0), stop=(ck == 1), skip_group_check=True,
                )
            res_sb = small.tile([B_CORE, H], f32, name="res_sb")
            nc.vector.tensor_add(res_sb[:], res_ps[:], res0_sb[:])

            nc.sync.dma_start(result, res_sb[:])
            psum_epi_cm.__exit__(None, None, None)

    nc.compile()
    return nc


def _get_nc():
    if "nc" not in _nc_cache:
        _nc_cache["nc"] = build_nc()
    return _nc_cache["nc"]


def build_in_maps(output, query, ref, Wq, bq, Wr, br, Wo, bo, V):
    output = np.asarray(output, dtype=np.float32)
    query = np.asarray(query, dtype=np.float32)
    ref = np.ascontiguousarray(np.asarray(ref, dtype=np.float32))
    Wq = np.asarray(Wq, np.float32)
    bq = np.asarray(bq, np.float32)
    Wr = np.asarray(Wr, np.float32)
    br = np.asarray(br, np.float32)
    Wo = np.asarray(Wo, np.float32)
    bo = np.asarray(bo, np.float32)
    V = np.ascontiguousarray(np.asarray(V, np.float32))

    # ---- host-side weight algebra (all O(H^2), negligible) ----
    # WrT_pk[p, l, m] = Wr[m, 2p+l]
    wrt_pk = np.ascontiguousarray(
        Wr.T.reshape(128, 2, H).astype(np.float32)
    )
    # q + bq + br per batch, chunked [128, 2, B]: value at (p, c, b) =
    # (query[b] @ Wq.T + bq + br)[c*128 + p]
    qproj = query @ Wq.T + bq + br  # [B, H]
    # MT[p, cm, n] = (Wo[:, H:] @ Wr)[n, cm*128+p]  (hi = cm*128+p chunking)
    M = Wo[:, H:] @ Wr  # [H, H]
    mt_pk = np.ascontiguousarray(
        M.T.reshape(2, 128, H).transpose(1, 0, 2).astype(np.float32)
    )
    # res0 = output @ WoA.T + (WoB @ br + bo)
    res0_full = output @ Wo[:, :H].T + (Wo[:, H:] @ br + bo)  # [B, H]

    in_maps = []
    for c in range(N_CORES):
        sl = slice(c * B_CORE, (c + 1) * B_CORE)
        bias_pk = np.ascontiguousarray(
            qproj[sl].T.reshape(2, 128, B_CORE).transpose(1, 0, 2).astype(np.float32)
        )
        in_maps.append(
            {
                "ref": ref[sl],
                "wrt_pk": wrt_pk,
                "bias_pk": bias_pk,
                "mt_pk": mt_pk,
                "res0": np.ascontiguousarray(res0_full[sl]),
                "V": V,
            }
        )
    return in_maps


def kernel(output, query, ref, Wq, bq, Wr, br, Wo, bo, V):
    from concourse.bass_utils import run_bass_kernel_spmd

    in_maps = build_in_maps(output, query, ref, Wq, bq, Wr, br, Wo, bo, V)
    nc = _get_nc()
    trace = bool(int(os.environ.get("KERNEL_TRACE", "0")))
    res = run_bass_kernel_spmd(nc, in_maps, list(range(N_CORES)), trace=trace)
    if trace:
        kernel.last_exec_time_ns = res.exec_time_ns
        kernel.last_profile = res
    out = np.concatenate([res.results[c]["result"] for c in range(N_CORES)], axis=0)
    return out.reshape(B, 1, H)


# revision 13
# speedup vs baseline: 1.1547x; 1.1547x over previous
"""Trainium2 Bass kernel for nn_Attention_49907519979595 (Bahdanau-style attention).

v5 design (v2 measured 165-172us, PE-bound: PE issue-time 155us =
transposes 55 + wsum 40 + r 31 + lgT 18 + misc; ACT 135 = tanh 71 +
PSUM copies 51 + exp/z 19).

Key change: ref streams from HBM as FP8 (SWDGE cast DMA), not bf16.
  - PE transposes operate on fp8 PAIRS viewed as uint16: 4 transposes per
    512-s tile instead of 8 (PE -27us), and the PSUM->SBUF copy volume
    halves to u16 (DVE-only, ~28-48us total, ACT keeps only tanh/exp).
  - The r-matmul moving operand is the packed refT: fp8 view of the u16
    transpose output with AP [Ki=128, l(stride 1), s(stride 2)], paired
    with a HOST-packed stationary WrT_pk[p, l, m] = Wr[m, 2p+l] so the
    DoubleRow contraction hi = 2*ki + l is consistent.
  - wsum runs DoubleRow too: stationary = e-column PAIRS (exp now emits
    fp8 e_col), moving = nat8[:, j:j+2, :] - 2 matmuls per tile (PE -13us).
  - All weight-only algebra is HOST-side numpy, passed as inputs:
    bias_sb = (Wq q^T + bq + br) chunks, MT = (WoB Wr)^T, and
    res0 = out_prev WoA^T + br WoB^T + bo.  Device epilogue is just
    uT = (wsum/Z)^T via tiny PE transposes + 2 matmuls + res0 add.

Engine budget targets: PE ~85 (r 29 + transposes 21 + lgT 18 + wsum 15),
ACT ~90 (tanh 71 + exp 19), DVE ~45-65, DMA-HBM ~94-116.
Precision: refT/Wr fp8 (logits-only, v2-proven), e_col fp8 (new, ~3%
weight noise -> ~1e-3 on expected; gate is 2e-2).
"""

import os
import sys

import numpy as np

sys.path.insert(0, "/opt/trn_rl_repo")

H = 256
B = 64
S = 4096
N_CORES = 8
B_CORE = B // N_CORES  # 8
S_TILE = 512
N_STILES = S // S_TILE  # 8 tiles per batch
NT = B_CORE * N_STILES  # 64 tiles
NJB = S // 128  # 32 s-chunks of 128 per batch

_nc_cache = {}


def build_nc():
    import concourse.bacc as bacc
    import concourse.tile as tile
    from concourse import masks, mybir

    f32 = mybir.dt.float32
    u16 = mybir.dt.uint16
    bf16 = mybir.dt.bfloat16
    f8 = mybir.dt.float8e4
    AF = mybir.ActivationFunctionType
    PM = mybir.MatmulPerfMode

    from concourse import bass_isa

    nc = bacc.Bacc("TRN2", debug=False)
    ref = nc.dram_tensor("ref", [B_CORE, S, H], f32, kind="ExternalInput").ap()
    # host-precomputed (see kernel()):
    wrt_pk = nc.dram_tensor("wrt_pk", [128, 2, H], f32, kind="ExternalInput").ap()
    bias_pk = nc.dram_tensor("bias_pk", [128, 2, B_CORE], f32, kind="ExternalInput").ap()
    mt_pk = nc.dram_tensor("mt_pk", [128, 2, H], f32, kind="ExternalInput").ap()
    res0 = nc.dram_tensor("res0", [B_CORE, H], f32, kind="ExternalInput").ap()
    V = nc.dram_tensor("V", [H], f32, kind="ExternalInput").ap()
    result = nc.dram_tensor("result", [B_CORE, H], f32, kind="ExternalOutput").ap()

    with tile.TileContext(nc) as tc:
        with (
            tc.tile_pool(name="const", bufs=1) as const,
            tc.tile_pool(name="natp", bufs=5) as natp,
            tc.tile_pool(name="reftp", bufs=3) as reftp,
            tc.tile_pool(name="tanhp", bufs=3) as tanhp,
            tc.tile_pool(name="small", bufs=6) as small,
        ):
            # Kick the first slice of batch 0 before anything else.
            nat0 = natp.tile([128, NJB, H], f8, name="nat", tag="nat")
            ref0 = ref[0].rearrange("(j p) h -> p j h", p=128)
            with nc.allow_low_precision(reason="fp8 ref stream"):
                nc.gpsimd.dma_start(nat0[:, 0:4, :], ref0[:, 0:4, :])

            # ---------------- prologue ----------------
            psum_pro_cm = tc.tile_pool(name="psum_pro", bufs=2, space="PSUM")
            psum_s = psum_pro_cm.__enter__()
            ident = const.tile([128, 128], f32, name="ident")
            masks.make_identity(nc, ident[:])
            ident_bf = const.tile([128, 128], bf16, name="ident_bf")
            nc.vector.tensor_copy(ident_bf[:], ident[:])

            # early HAM warm-up while weight DMAs land
            warm0_ps = psum_s.tile([128, 128], bf16, name="warm0_ps", tag="warm0")
            for _ in range(40):
                nc.tensor.transpose(warm0_ps[:], ident_bf[:], ident_bf[:])

            def load(shape, src_ap, name, eng):
                t = const.tile(shape, f32, name=name)
                eng.dma_start(t[:], src_ap)
                return t

            wrtpk_sb = load([128, 2, H], wrt_pk, "wrtpk_sb", nc.sync)
            wrtpk_f8 = const.tile([128, 2, H], f8, name="wrtpk_f8")
            with nc.allow_low_precision(reason="fp8 r-matmul; logits-only"):
                nc.vector.tensor_copy(wrtpk_f8[:], wrtpk_sb[:])
            bias_sb = load([128, 2, B_CORE], bias_pk, "bias_sb", nc.scalar)
            MT = load([128, 2, H], mt_pk, "MT", nc.sync)
            res0_sb = load([B_CORE, H], res0, "res0_sb", nc.scalar)
            V_f32 = load([128, 2], V.rearrange("(c p) -> p c", p=128), "V_f32", nc.scalar)
            V_col = const.tile([128, 2], f8, name="V_col")
            with nc.allow_low_precision(reason="fp8 logits; softmax-only"):
                nc.vector.tensor_copy(V_col[:], V_f32[:])

            # per-batch streaming outputs
            acc_bh = const.tile([B_CORE, H], f32, name="acc_bh")
            zcol_all = const.tile([128, B_CORE], f32, name="zcol_all")
            z_sb = const.tile([1, B_CORE], f32, name="z_sb")

            psum_pro_cm.__exit__(None, None, None)
            # main-loop PSUM: r 2x2 + refT_pk 2 + lgT 1 + wsum 1 = 8 banks
            psum_r_cm = tc.tile_pool(name="psum_r", bufs=2, space="PSUM")
            psum_r = psum_r_cm.__enter__()
            psum_t_cm = tc.tile_pool(name="psum_t", bufs=2, space="PSUM")
            psum_t = psum_t_cm.__enter__()
            psum_lg_cm = tc.tile_pool(name="psum_lg", bufs=1, space="PSUM")
            psum_lg = psum_lg_cm.__enter__()
            psum_ws_cm = tc.tile_pool(name="psum_ws", bufs=1, space="PSUM")
            psum_ws = psum_ws_cm.__enter__()
            wsum_acc = psum_ws.tile([1, 256], f32, name="wsum_acc", tag="wsacc")

            # ---------------- main loop (software-pipelined emission) ----------------
            st = {}

            def emit_load(b, slices=(NJB,)):
                nat = natp.tile([128, NJB, H], f8, name="nat", tag="nat")
                src = ref[b].rearrange("(j p) h -> p j h", p=128)
                j0 = 0
                with nc.allow_low_precision(reason="fp8 ref stream"):
                    for js in slices:
                        nc.gpsimd.dma_start(
                            nat[:, j0 : j0 + js, :], src[:, j0 : j0 + js, :]
                        )
                        j0 += js
                st[("nat", b)] = nat

            for a, bnd in ((4, 8), (8, 16), (16, 32)):
                with nc.allow_low_precision(reason="fp8 ref stream"):
                    nc.gpsimd.dma_start(nat0[:, a:bnd, :], ref0[:, a:bnd, :])
            st[("nat", 0)] = nat0
            emit_load(1, slices=(16, 16))

            def stage_load(v):
                b, t = divmod(v, N_STILES)
                if t == 0 and b + 2 < B_CORE:
                    emit_load(b + 2)

            # packed transposes: nat8 pair-chunks as u16 [128, 128] each
            def stage_transpose(v):
                b, t = divmod(v, N_STILES)
                nat = st[("nat", b)]
                rt_ps = psum_t.tile([128, 4, 128], bf16, name="rt_ps", tag="rtps")
                for j in range(4):
                    nc.tensor.transpose(
                        rt_ps[:, j, :],
                        nat[:, t * 4 + j, :].bitcast(bf16),
                        ident_bf[:],
                    )
                st[("rtps", v)] = rt_ps

            def stage_copy(v):
                rt_ps = st.pop(("rtps", v))
                refT_pk = reftp.tile([128, 4, 128], bf16, name="refT_pk", tag="refT")
                nc.vector.tensor_copy(refT_pk[:], rt_ps[:])
                st[("refT", v)] = refT_pk

            def stage_r(v):
                # fires on odd v; emits BOTH tiles of the pair hh-outer so
                # one DoubleRow LDWEIGHTS per hh serves two matmuls, but the
                # two tiles get SEPARATE double-buffered PSUM tiles so the
                # next pair's r doesn't WAR-wait on this pair's full tanh.
                if v % 2 == 0:
                    return
                r0 = psum_r.tile([128, 2, S_TILE], f32, name="r_ps", tag="rps")
                r1 = psum_r.tile([128, 2, S_TILE], f32, name="r_ps", tag="rps")
                rps = [r0, r1]
                refTs = [st.pop(("refT", v - 1)), st.pop(("refT", v))]
                movs = [
                    t[:].bitcast(f8).rearrange("p j (s l) -> p l j s", l=2)
                    for t in refTs
                ]
                for hh in range(2):
                    for ti in range(2):
                        nc.tensor.matmul(
                            rps[ti][:, hh, :],
                            wrtpk_f8[:, :, hh * 128 : (hh + 1) * 128],
                            movs[ti][:],
                            start=True,
                            stop=True,
                            perf_mode=PM.DoubleRow,
                        )
                st[("rps", v - 1)] = r0
                st[("rps", v)] = r1

            def stage_tanh(v):
                b, t = divmod(v, N_STILES)
                r_ps = st.pop(("rps", v))
                tanh_sb = tanhp.tile(
                    [128, 2, S_TILE], f8, name="tanh_sb", tag="tanh",
                )
                with nc.allow_low_precision(reason="fp8 tanh; softmax-only"):
                    for hh in range(2):
                        nc.scalar.activation(
                            tanh_sb[:, hh, :],
                            r_ps[:, hh, :],
                            AF.Tanh,
                            bias=bias_sb[:, hh, b : b + 1],
                        )
                st[("tanh", v)] = tanh_sb

            def stage_lg(v):
                tanh_sb = st.pop(("tanh", v))
                ti = v % 2
                if ti == 0:
                    st[("lgT", v // 2)] = psum_lg.tile(
                        [128, 2 * 4], f32, name="lgT_ps", tag="lgT"
                    )
                lgT_ps = st[("lgT", v // 2)]
                for c in range(4):
                    col = ti * 4 + c
                    for hh in range(2):
                        nc.tensor.matmul(
                            lgT_ps[:, col : col + 1],
                            tanh_sb[:, hh, c * 128 : (c + 1) * 128],
                            V_col[:, hh : hh + 1],
                            start=(hh == 0),
                            stop=(hh == 1),
                        )

            def stage_exp(v):
                if v % 2 == 0:
                    return
                b, t = divmod(v, N_STILES)
                lgT_ps = st.pop(("lgT", v // 2))
                e_col = small.tile([128, 2 * 4], f8, name="e_col", tag="e_col", bufs=3)
                zt = small.tile([128, 1], f32, name="zt", tag="zt", bufs=4)
                with nc.allow_low_precision(reason="fp8 softmax weights"):
                    nc.scalar.activation(e_col[:], lgT_ps[:], AF.Exp, accum_out=zt[:])
                if t == 1:
                    nc.vector.tensor_copy(zcol_all[:, b : b + 1], zt[:])
                else:
                    nc.vector.tensor_add(
                        zcol_all[:, b : b + 1], zcol_all[:, b : b + 1], zt[:]
                    )
                st[("e_col", v - 1)] = e_col
                st[("e_col", v)] = e_col

            def stage_wsum(v):
                b, t = divmod(v, N_STILES)
                e_col = st.pop(("e_col", v))
                nat = st[("nat", b)]
                slot = wsum_acc[:]
                for c in range(4):
                    j = t * 4 + c
                    nc.tensor.matmul(
                        slot,
                        e_col[:, (v % 2) * 4 + c : (v % 2) * 4 + c + 1],
                        nat[:, j, :],
                        start=(j == 0),
                        stop=(j == NJB - 1),
                        skip_group_check=True,
                    )
                if t == N_STILES - 1:
                    ws_sb = small.tile([1, H], f32, name="ws_sb", tag="ws_sb", bufs=2)
                    nc.vector.tensor_copy(ws_sb[:], slot)
                    nc.sync.dma_start(acc_bh[b : b + 1, :], ws_sb[:])
                    zred = small.tile([128, 1], f32, name="zred", tag="zred", bufs=2)
                    nc.gpsimd.partition_all_reduce(
                        zred[:], zcol_all[:, b : b + 1], channels=128,
                        reduce_op=bass_isa.ReduceOp.add,
                    )
                    nc.vector.tensor_copy(z_sb[0:1, b : b + 1], zred[0:1, :])
                    st.pop(("nat", b))

            # exp emitted before tanh AND before lg: exp(pair p) must enter
            # the ACT queue ahead of newer tanhs (latency), and before
            # lg(pair p+1) allocates the bufs=1 lgT slot (WAR tracking).
            STAGES = [
                (stage_load, 0),
                (stage_copy, 1),
                (stage_transpose, 0),
                (stage_r, 2),
                (stage_exp, 5),
                (stage_tanh, 3),
                (stage_lg, 4),
                (stage_wsum, 6),
            ]
            LOOKAHEAD = 7
            for step in range(NT + LOOKAHEAD):
                for fn, off in STAGES:
                    w = step - off
                    if 0 <= w < NT:
                        fn(w)

            psum_ws_cm.__exit__(None, None, None)
            psum_lg_cm.__exit__(None, None, None)
            psum_t_cm.__exit__(None, None, None)
            psum_r_cm.__exit__(None, None, None)

            # ---------------- epilogue ----------------
            psum_epi_cm = tc.tile_pool(name="psum_epi", bufs=2, space="PSUM")
            psum_s = psum_epi_cm.__enter__()

            # Z: z_sb [1, 8] -> zrow [8, 1] via one PE transpose
            zt_ps = psum_s.tile([B_CORE, 1], f32, name="zt_ps", tag="ps")
            nc.tensor.transpose(zt_ps[:], z_sb[:], ident[0:1, 0:1])
            zrow = small.tile([B_CORE, 1], f32, name="zrow")
            nc.vector.tensor_copy(zrow[:], zt_ps[:])
            rz = small.tile([B_CORE, 1], f32, name="rz")
            nc.vector.reciprocal(rz[:], zrow[:])

            # u = acc / Z, then transpose to [128, 2, B]
            u_bh = small.tile([B_CORE, H], f32, name="u_bh")
            nc.vector.tensor_scalar_mul(u_bh[:], acc_bh[:], rz[:])
            uT = small.tile([128, 2, B_CORE], f32, name="uT")
            for c in range(2):
                ut_ps = psum_s.tile([128, B_CORE], f32, name="ut_ps", tag="ps")
                nc.tensor.transpose(
                    ut_ps[:], u_bh[:, c * 128 : (c + 1) * 128], ident[:B_CORE, :B_CORE]
                )
                nc.vector.tensor_copy(uT[:, c, :], ut_ps[:])

            # res = res0 + uT.T @ MT
            res_ps = psum_s.tile([B_CORE, H], f32, name="res_ps", tag="ps")
            for ck in range(2):
                nc.tensor.matmul(
                    res_ps[:], uT[:, ck, :], MT[:, ck, :],
                    start=(ck == 0), stop=(ck == 1), skip_group_check=True,
                )
            res_sb = small.tile([B_CORE, H], f32, name="res_sb")
            nc.vector.tensor_add(res_sb[:], res_ps[:], res0_sb[:])

            nc.sync.dma_start(result, res_sb[:])
            psum_epi_cm.__exit__(None, None, None)

    nc.compile()
    return nc


def _get_nc():
    if "nc" not in _nc_cache:
        _nc_cache["nc"] = build_nc()
    return _nc_cache["nc"]


def build_in_maps(output, query, ref, Wq, bq, Wr, br, Wo, bo, V):
    output = np.asarray(output, dtype=np.float32)
    query = np.asarray(query, dtype=np.float32)
    ref = np.ascontiguousarray(np.asarray(ref, dtype=np.float32))
    Wq = np.asarray(Wq, np.float32)
    bq = np.asarray(bq, np.float32)
    Wr = np.asarray(Wr, np.float32)
    br = np.asarray(br, np.float32)
    Wo = np.asarray(Wo, np.float32)
    bo = np.asarray(bo, np.float32)
    V = np.ascontiguousarray(np.asarray(V, np.float32))

    # ---- host-side weight algebra (all O(H^2), negligible) ----
    # WrT_pk[p, l, m] = Wr[m, 2p+l]
    wrt_pk = np.ascontiguousarray(
        Wr.T.reshape(128, 2, H).astype(np.float32)
    )
    # q + bq + br per batch, chunked [128, 2, B]: value at (p, c, b) =
    # (query[b] @ Wq.T + bq + br)[c*128 + p]
    qproj = query @ Wq.T + bq + br  # [B, H]
    # MT[p, cm, n] = (Wo[:, H:] @ Wr)[n, cm*128+p]  (hi = cm*128+p chunking)
    M = Wo[:, H:] @ Wr  # [H, H]
    mt_pk = np.ascontiguousarray(
        M.T.reshape(2, 128, H).transpose(1, 0, 2).astype(np.float32)
    )
    # res0 = output @ WoA.T + (WoB @ br + bo)
    res0_full = output @ Wo[:, :H].T + (Wo[:, H:] @ br + bo)  # [B, H]

    in_maps = []
    for c in range(N_CORES):
        sl = slice(c * B_CORE, (c + 1) * B_CORE)
        bias_pk = np.ascontiguousarray(
            qproj[sl].T.reshape(2, 128, B_CORE).transpose(1, 0, 2).astype(np.float32)
        )
        in_maps.append(
            {
                "ref": ref[sl],
                "wrt_pk": wrt_pk,
                "bias_pk": bias_pk,
                "mt_pk": mt_pk,
                "res0": np.ascontiguousarray(res0_full[sl]),
                "V": V,
            }
        )
    return in_maps


def kernel(output, query, ref, Wq, bq, Wr, br, Wo, bo, V):
    from concourse.bass_utils import run_bass_kernel_spmd

    in_maps = build_in_maps(output, query, ref, Wq, bq, Wr, br, Wo, bo, V)
    nc = _get_nc()
    trace = bool(int(os.environ.get("KERNEL_TRACE", "0")))
    res = run_bass_kernel_spmd(nc, in_maps, list(range(N_CORES)), trace=trace)
    if trace:
        kernel.last_exec_time_ns = res.exec_time_ns
        kernel.last_profile = res
    out = np.concatenate([res.results[c]["result"] for c in range(N_CORES)], axis=0)
    return out.reshape(B, 1, H)
